# revision 7
# baseline (speedup 1.0000x reference)
"""DAS dual-speed-of-sound beamforming kernel for 8 Trainium2 NeuronCores.

Computation: out[h,w] = mean_n sino[n, clip(round(((dtx-db+re-dd)/v0 + db/v1)/Ts))]

Strategy (sharding_hint): shard the transducer axis N=256 across 8 cores
(32 each). Each core streams its dist_tx/dist_body shard (16MB), computes
time-of-flight indices on VectorE (two fused affine ops + round-to-nearest-even
int16 convert, which bit-matches jnp.round), gathers from its sinogram rows
with GpSimd ap_gather, and accumulates partial sums over its transducers.
Host sums the 8 x 8-group partials and divides by N.

ap_gather semantics force one index list per 16-partition group, so each of
the 8 groups processes one transducer per pass (16x redundant rows, identical
values). 4 passes x 8 groups cover the 32 transducers. Both reference clip
boundaries land on zeroed samples (sino[:,0] = sino[:,-1] = 0), and the ucode
clamps negative indices to 0, so a zero-padded table gives exact clip
semantics with no clamp instructions.
"""

import sys

sys.path.insert(0, "/opt/trn_rl_repo")

import numpy as np

import concourse.bass as bass  # noqa: F401  (bass must import before tile)
import concourse.tile as tile
from concourse import bacc, mybir
from concourse.bass_utils import run_bass_kernel_spmd

# Problem geometry (fixed by the nn.Module)
N = 256          # transducers
H = 256
W = 256
T = 2048         # time samples
T_SAMPLE = 2.5e-8
NCORES = 8
NSH = N // NCORES          # 32 transducers per core
PIX = H * W                # 65536 pixels
NA = 4                     # transducer assignments (4 x 8 groups = 32)
NCHUNK = 8
CHUNK = PIX // NCHUNK      # 8192 pixels per gather instruction
S = CHUNK // 16            # 512 idx values per partition (wrapped layout)

_BUILD_CACHE = {}


def _split_const(v):
    """Dekker 12-bit split of an f32 constant, computed host-side in f32."""
    f = np.float32
    v = f(v)
    c = f(f(v) * f(4097.0))
    hi = f(c - f(c - v))
    lo = f(v - hi)
    return float(hi), float(lo)


def _build(v0: float, v1: float, ts: float, re_m_dd: float, pad_t: int):
    """Compile the per-core SPMD Bass kernel with the scalars baked in."""
    key = (v0, v1, ts, re_m_dd, pad_t)
    if key in _BUILD_CACHE:
        return _BUILD_CACHE[key]

    f32 = mybir.dt.float32
    i16 = mybir.dt.int16
    nc = bacc.Bacc("TRN2", target_bir_lowering=False, debug=False,
                   enable_asserts=False)
    tx_d = nc.dram_tensor("txs", [NA, NCHUNK, 128, S], f32,
                          kind="ExternalInput").ap()
    bd_d = nc.dram_tensor("bds", [NA, NCHUNK, 128, S], f32,
                          kind="ExternalInput").ap()
    sino_d = nc.dram_tensor("sino_rep", [NA * 128, pad_t], f32,
                            kind="ExternalInput").ap()
    out_d = nc.dram_tensor("out", [NCHUNK, 8, CHUNK], f32,
                           kind="ExternalOutput").ap()

    with tile.TileContext(nc) as tc:
        with tc.tile_pool(name="data", bufs=1) as dpool, \
             tc.tile_pool(name="io", bufs=3) as iopool, \
             tc.tile_pool(name="tmp", bufs=1) as tpool, \
             tc.tile_pool(name="idxp", bufs=3) as ipool, \
             tc.tile_pool(name="gat", bufs=2) as gpool, \
             tc.tile_pool(name="acc", bufs=1) as apool:
            # Per-assignment sinogram tables, resident for the whole kernel.
            data_t = []
            for a in range(NA):
                dt_ = dpool.tile([128, pad_t], f32, tag=f"data{a}")
                nc.sync.dma_start(dt_[:], sino_d[128 * a:128 * (a + 1), :])
                data_t.append(dt_)

            for i in range(NCHUNK):
                acc = apool.tile([128, CHUNK], f32)
                for a in range(NA):
                    tx_t = iopool.tile([128, S], f32, tag="tx")
                    nc.sync.dma_start(tx_t[:], tx_d[a, i])
                    bd_t = iopool.tile([128, S], f32, tag="bd")
                    nc.sync.dma_start(bd_t[:], bd_d[a, i])

                    # Bit-exact emulation of the reference's f32 division
                    # chain (Dekker-product Newton correction; verified
                    # 0/16.7M rounding flips vs jnp on the real geometry):
                    #   idx = round(((tx - bd + re - dd)/v0 + bd/v1)/Ts)
                    MUL = mybir.AluOpType.mult
                    ADD = mybir.AluOpType.add
                    SUB = mybir.AluOpType.subtract

                    def ediv(x_ap, v, tagp):
                        v = np.float32(v)
                        inv = float(np.float32(1.0) / v)
                        vh, vl = _split_const(v)
                        tl = [tpool.tile([128, S], f32, tag=f"{tagp}{k}",
                                         name=f"{tagp}{k}")
                              for k in range(6)]
                        d, cc, dh, dl, p, e1 = tl
                        nc.vector.tensor_scalar(d[:], x_ap, inv, None, MUL)
                        nc.vector.tensor_scalar(cc[:], d[:], 4097.0, None, MUL)
                        # dh = cc - (cc - d); dl = d - dh
                        nc.vector.tensor_sub(dh[:], cc[:], d[:])
                        nc.vector.tensor_sub(cc[:], cc[:], dh[:])
                        nc.vector.tensor_sub(dl[:], d[:], cc[:])
                        nc.vector.tensor_scalar(p[:], d[:], float(v), None, MUL)
                        # e1 = dh*vh - p  (dh now lives in cc)
                        nc.vector.scalar_tensor_tensor(
                            e1[:], cc[:], vh, p[:], MUL, SUB)
                        if vl != 0.0:
                            m1 = tpool.tile([128, S], f32, tag=f"{tagp}m1")
                            nc.vector.tensor_scalar(m1[:], cc[:], vl, None, MUL)
                            nc.vector.scalar_tensor_tensor(
                                m1[:], dl[:], vh, m1[:], MUL, ADD)
                            nc.vector.tensor_add(e1[:], e1[:], m1[:])
                            nc.vector.tensor_scalar(m1[:], dl[:], vl, None, MUL)
                            nc.vector.tensor_add(e1[:], e1[:], m1[:])
                        else:
                            nc.vector.scalar_tensor_tensor(
                                e1[:], dl[:], vh, e1[:], MUL, ADD)
                        # e = (x - p) - err;  result = d + e*inv
                        nc.vector.tensor_sub(p[:], x_ap, p[:])
                        nc.vector.tensor_sub(p[:], p[:], e1[:])
                        nc.vector.scalar_tensor_tensor(
                            d[:], p[:], inv, d[:], MUL, ADD)
                        return d

                    q = tpool.tile([128, S], f32, tag="q")
                    nc.vector.tensor_sub(q[:], tx_t[:], bd_t[:])
                    if re_m_dd != 0.0:
                        nc.vector.tensor_scalar(
                            q[:], q[:], float(re_m_dd), None, ADD)
                    r_t = ediv(q[:], v0, "dv0_")
                    s_t = ediv(bd_t[:], v1, "dv1_")
                    nc.vector.tensor_add(r_t[:], r_t[:], s_t[:])
                    x_t = ediv(r_t[:], ts, "dts_")
                    idx_t = ipool.tile([128, S], i16, tag="idx")
                    nc.vector.tensor_copy(idx_t[:], x_t[:])

                    g_t = gpool.tile([128, CHUNK], f32, tag="g")
                    nc.gpsimd.ap_gather(
                        g_t[:], data_t[a][:], idx_t[:],
                        channels=128, num_elems=pad_t, d=1, num_idxs=CHUNK)

                    if a == 0:
                        nc.vector.tensor_copy(acc[:], g_t[:])
                    else:
                        nc.vector.tensor_add(acc[:], acc[:], g_t[:])

                # Emit one (identical) row per 16-partition group; host sums
                # the 8 group partials.
                for g in range(8):
                    nc.sync.dma_start(out_d[i, g], acc[16 * g:16 * g + 1, :])

    nc.compile()
    _BUILD_CACHE[key] = nc
    return nc


def kernel(sinogram, v0, v1, d_delay, ring_error, dist_tx, dist_body):
    sinogram = np.asarray(sinogram, dtype=np.float32)
    dist_tx = np.asarray(dist_tx, dtype=np.float32)
    dist_body = np.asarray(dist_body, dtype=np.float32)
    v0 = float(np.asarray(v0))
    v1 = float(np.asarray(v1))
    d_delay = float(np.asarray(d_delay))
    ring_error = float(np.asarray(ring_error))

    # Index model (device computes it with bit-exact f32 division emulation):
    #   idx = round(((dist_tx - dist_body + re - dd)/v0 + dist_body/v1)/Ts)
    a_s = 1.0 / (v0 * T_SAMPLE)
    b_s = 1.0 / (v1 * T_SAMPLE) - 1.0 / (v0 * T_SAMPLE)
    c_s = (ring_error - d_delay) / (v0 * T_SAMPLE)

    # Upper bound on the pre-round value (interval arithmetic on the inputs)
    # to size the zero-padded gather table. Out-of-range-high gathers must
    # stay inside the table; they return 0 = sino[:, -1] after zeroing, which
    # is exactly the reference's clip behaviour. Negative indices are clamped
    # to 0 by the ap_gather ucode = reference clip-low (sino[:, 0] is zeroed).
    tx_lo, tx_hi = float(dist_tx.min()), float(dist_tx.max())
    bd_lo, bd_hi = float(dist_body.min()), float(dist_body.max())
    hi = (max(a_s * tx_lo, a_s * tx_hi)
          + max(b_s * bd_lo, b_s * bd_hi) + c_s + 1.0)
    lo = (min(a_s * tx_lo, a_s * tx_hi)
          + min(b_s * bd_lo, b_s * bd_hi) + c_s - 1.0)
    assert lo > -32000.0, f"index lower bound {lo} out of int16 range"
    assert hi < 32000.0, f"index upper bound {hi} out of int16 range"
    pad_t = max(T + 256, int(np.ceil(hi)) + 64)
    pad_t = min((pad_t + 127) // 128 * 128, 32768)

    # mode == 'zero': zero first/last time samples; zero-pad the table.
    sino_p = np.zeros((N, pad_t), np.float32)
    sino_p[:, :T] = sinogram
    sino_p[:, 0] = 0.0
    sino_p[:, T - 1] = 0.0

    nc = _build(v0, v1, T_SAMPLE, ring_error - d_delay, pad_t)

    # Host-side marshaling into device layouts.
    # txs[a, i, 16g+j, s] = dist_tx[32c + 8a + g, pix], pix = 8192i + 512j + s
    in_maps = []
    for c in range(NCORES):
        txc = dist_tx[NSH * c:NSH * (c + 1)].reshape(NA, 8, NCHUNK, 16, S)
        bdc = dist_body[NSH * c:NSH * (c + 1)].reshape(NA, 8, NCHUNK, 16, S)
        txs = np.ascontiguousarray(txc.transpose(0, 2, 1, 3, 4)
                                   ).reshape(NA, NCHUNK, 128, S)
        bds = np.ascontiguousarray(bdc.transpose(0, 2, 1, 3, 4)
                                   ).reshape(NA, NCHUNK, 128, S)
        # sino_rep[128a + 16g + j] = sino_p[32c + 8a + g]
        rep = np.repeat(sino_p[NSH * c:NSH * (c + 1)], 16, axis=0)
        in_maps.append({"txs": txs, "bds": bds, "sino_rep": rep})

    res = run_bass_kernel_spmd(nc, in_maps, core_ids=list(range(NCORES)))

    # Host reduction: sum the 8 group rows per chunk per core, un-permute the
    # wrapped pixel order (pixel = 8192i + 512*(u%16) + u//16), sum cores.
    total = np.zeros(PIX, np.float64)
    for c in range(NCORES):
        o = res.results[c]["out"]          # [NCHUNK, 8, CHUNK]
        chunks = o.sum(axis=1, dtype=np.float64)   # [NCHUNK, CHUNK]
        for i in range(NCHUNK):
            total[CHUNK * i:CHUNK * (i + 1)] += (
                chunks[i].reshape(S, 16).T.reshape(-1))
    out = (total / N).astype(np.float32).reshape(H, W)
    return out
